# revision 20
# baseline (speedup 1.0000x reference)
"""CrossFocusedLinearAttention Trainium2 kernel (bf16 rework).

Per-core computation (1 batch item per NeuronCore, 8 cores):
  q = relu(query @ Wq)/s; k = relu(key_in @ Wk)/s   (s = softplus(scale), folded
  into Wq/Wk columns on host)
  focus(x) = x^3 * ||x|| / ||x^3||  per token
  per head: kv = k_f^T v ; t = q_f . ksum; x = (q_f @ kv) / (t + eps)
  out = x @ Wp + bp

Algebraic restructure vs the fp32r baseline (all matmuls bf16, 1 cyc/row):
  - v-projection reassociated:  kv = u3k_rk^T (vin Wv)  ->  KVrawT = vin^T u3rk
    (4 wide MMs/tile replacing vproj + narrow kv MMs), then per-head
    kv_bdT[d,c] = Wv[:,hs]^T KVrawT[:,hs] at the phase transition.
  - Wp folded in:  M[hs,:] = kv_bdT^T Wp[hs,:]  (dense [C,C]), so phase 2 is
    out^T = sum_nt M[nt,et]^T (g * u3q)[nt] -- the separate x matmul is gone.
  - ksum computed as a single [1,C] row MM per tile (ones stationary), then
    transposed once via a DRAM bounce into column layout for m8rep.
  - t replicated per channel:  t_rep = m8rep_diag[nt]^T u3q[nt], one MM per
    tile (m8rep_diag = blockdiag mask * ksum per partition), g = 1/(t+eps)
    via ACT identity+bias then DVE reciprocal; applied to u3q before the M MM.

Phase 1 per 128-token tile: 4 kproj + 4 KVrawT + 1 ksum = 9 MMs (512-free).
Phase 2 per 512-token chunk: 16 qproj + 4 t_rep + 16 out = 36 MMs.
"""

import os
import sys

import numpy as np

sys.path.insert(0, "/opt/trn_rl_repo")

P = 128
C = 512
N = 4096
CT = C // P            # 4 channel tiles
NH = 8                 # heads
HD = C // NH           # 64 head dim
JBLK = 512             # phase-1 token chunk
JC = N // JBLK         # 8
JSUB = JBLK // P       # 4 token tiles per chunk
IBLK = 512             # phase-2 token chunk
ICN = N // IBLK        # 8
EPS = 1e-6
NCORES = 8
LAG = 10               # phase-1 software-pipeline lag (sub-tiles)

_CACHE = {}


def _build_nc():
    import concourse.mybir as mybir
    import concourse.tile as tile
    from concourse import bacc
    from contextlib import ExitStack

    f32 = mybir.dt.float32
    bf16 = mybir.dt.bfloat16
    AF = mybir.ActivationFunctionType
    OP = mybir.AluOpType

    # Pin every ACTIVATE to natural_log_exp_and_others (contains relu,
    # square, ln, exp, identity, copy) — the default greedy set chooser
    # flip-flops between two tables, costing ~1.3us per reload.
    class _BaccOneActTable(bacc.Bacc):
        def insert_act_table_loads(self):
            import bass_rust as _br
            from concourse.hw_specs import get_activation_tables
            has_activation = any(
                isinstance(i, mybir.InstActivation)
                for b in self.main_func.blocks
                for i in b.instructions
            )
            if not has_activation:
                return
            tables = [
                (n, (s if n == "natural_log_exp_and_others" else set()))
                for n, s in get_activation_tables(self.m.arch).items()
            ]
            _br.insert_act_table_loads(self, tables)

    nc = _BaccOneActTable("TRN2", target_bir_lowering=False, debug=False)

    qT = nc.declare_dram_parameter("qT", [C, N], bf16, isOutput=False)
    kT = nc.declare_dram_parameter("kT", [C, N], bf16, isOutput=False)
    vN = nc.declare_dram_parameter("vN", [N, C], bf16, isOutput=False)
    Wq = nc.declare_dram_parameter("Wq", [C, C], bf16, isOutput=False)
    Wk = nc.declare_dram_parameter("Wk", [C, C], bf16, isOutput=False)
    Wv = nc.declare_dram_parameter("Wv", [C, C], bf16, isOutput=False)
    Wp = nc.declare_dram_parameter("Wp", [C, C], bf16, isOutput=False)
    bp_col = nc.declare_dram_parameter("bp_col", [P, CT], f32, isOutput=False)
    ksb = nc.declare_dram_parameter("ksb", [1, C], f32, isOutput=True)
    outT = nc.declare_dram_parameter("outT", [C, N], f32, isOutput=True)

    # DRAM views
    qT_v = qT.rearrange("(t p) n -> p t n", p=P)
    kT_v = kT.rearrange("(t p) n -> p t n", p=P)
    vN_v = vN.rearrange("(j s p) c -> p j s c", s=JSUB, p=P)
    outT_v = outT.rearrange("(t p) n -> p t n", p=P)
    Wq_v = Wq.rearrange("(t p) n -> p t n", p=P)
    Wk_v = Wk.rearrange("(t p) n -> p t n", p=P)
    Wv_v = Wv.rearrange("(t p) n -> p t n", p=P)
    Wp_v = Wp.rearrange("(t p) n -> p t n", p=P)
    ksb_colv = ksb.rearrange("o (t p) -> (o p) t", p=P)

    with ExitStack() as ctx:
        tc = ctx.enter_context(tile.TileContext(nc))

        # ---------- persistent SBUF ----------
        wpool = ctx.enter_context(tc.tile_pool(name="weights", bufs=1))
        wk = wpool.tile([P, CT, C], bf16, tag="wk")
        wq = wpool.tile([P, CT, C], bf16, tag="wq")
        wv = wpool.tile([P, CT, C], bf16, tag="wv")
        wp = wpool.tile([P, CT, C], bf16, tag="wp")
        M_sb = wpool.tile([P, CT, C], bf16, tag="Msb")
        m8rep = wpool.tile([P, CT, P], bf16, tag="m8rep")
        KVr_sb = wpool.tile([P, CT, C], bf16, tag="kvrsb")
        kvbd_sb = wpool.tile([P, CT, HD], bf16, tag="kvbdsb")
        bp_sb = wpool.tile([P, CT], f32, tag="bp")
        ones_sb = wpool.tile([P, 1], bf16, tag="ones")
        ones_hd = wpool.tile([P, HD], bf16, tag="oneshd")
        ceps = wpool.tile([P, 1], f32, tag="ceps")
        krow_sb = wpool.tile([1, C], f32, tag="krow")
        kcol_sb = wpool.tile([P, CT], f32, tag="kcol")
        # phase-1-critical load only; the rest mid-phase-1.
        nc.sync.dma_start(wk[:], Wk_v[:])
        nc.vector.memset(ceps[:], EPS)
        nc.vector.memset(ones_sb[:], 1.0)
        nc.vector.memset(ones_hd[:], 1.0)

        # kv psum held across all of phase 1 (KVrawT 4 banks + ksum 1)
        kvstack = ExitStack()
        kvpool = kvstack.enter_context(
            tc.tile_pool(name="kvps", bufs=1, space="PSUM"))
        kvr = [kvpool.tile([P, C], f32, tag=f"kvr{t}", name=f"kvr{t}")
               for t in range(CT)]
        ksum_ps = kvpool.tile([1, C], f32, tag="ksum")

        # phase-2 SBUF pools + helpers (entered early: qproj(0) is emitted
        # inside phase 1, on the kproj psum pool, to cover the kvraw drain)
        ldq = ctx.enter_context(tc.tile_pool(name="qld", bufs=3))
        wkq = ctx.enter_context(tc.tile_pool(name="p2work", bufs=3))
        u3qp = ctx.enter_context(tc.tile_pool(name="u3q", bufs=13))
        gup = ctx.enter_context(tc.tile_pool(name="gu3q", bufs=13))
        gwp = ctx.enter_context(tc.tile_pool(name="p2g", bufs=3))
        osp = ctx.enter_context(tc.tile_pool(name="osb", bufs=3))
        qts = {}

        def dma_q(ic):
            isl = slice(ic * IBLK, (ic + 1) * IBLK)
            qts[ic] = ldq.tile([P, CT, IBLK], bf16, tag="qld", name="qld")
            nc.sync.dma_start(qts[ic][:], qT_v[:, :, isl])

        def emit_qproj(ic, pool, ptag):
            qtile = qts.pop(ic)
            u3qs = []
            for nt in range(CT):
                qps = pool.tile([P, IBLK], f32, tag=ptag, name="qps")
                for ct in range(CT):
                    nc.tensor.matmul(
                        qps[:], wq[:, ct, nt * P:(nt + 1) * P],
                        qtile[:, ct, :],
                        start=(ct == 0), stop=(ct == CT - 1))
                rlu = wkq.tile([P, IBLK], bf16, tag="rluq", name="rluq")
                nc.scalar.activation(rlu[:], qps[:], AF.Relu)
                u2q = wkq.tile([P, IBLK], bf16, tag="u2q", name="u2q")
                nc.gpsimd.tensor_tensor(u2q[:], rlu[:], rlu[:], OP.mult)
                u3q = u3qp.tile([P, IBLK], bf16, tag="u3q", name="u3q")
                nc.vector.tensor_tensor(u3q[:], u2q[:], rlu[:], OP.mult)
                u3qs.append(u3q)
            return u3qs

        # ================= PHASE 1: k/v -> KVrawT, ksum =================
        with ExitStack() as p1:
            kpp = p1.enter_context(
                tc.tile_pool(name="p1kproj", bufs=3, space="PSUM"))
            ldp = p1.enter_context(tc.tile_pool(name="p1ld", bufs=4))
            wkp = p1.enter_context(tc.tile_pool(name="p1work", bufs=3))
            u3rkp = p1.enter_context(tc.tile_pool(name="u3rk", bufs=LAG + 2))
            smp = p1.enter_context(tc.tile_pool(name="p1small", bufs=LAG + 3))

            pend = []      # (u3rk_tile, vtile, jj) awaiting KVrawT emission
            nkv = [0]      # KVrawT MMs emitted (for start/stop flags)
            NKV_TOTAL = JC * JSUB

            def emit_kvraw():
                u3rk, vtile, jj = pend.pop(0)
                first = nkv[0] == 0
                last = nkv[0] == NKV_TOTAL - 1
                nkv[0] += 1
                for dt in range(CT):
                    nc.tensor.matmul(
                        kvr[dt][:], vtile[:, jj, dt * P:(dt + 1) * P],
                        u3rk[:], start=first, stop=last)
                nc.tensor.matmul(
                    ksum_ps[:], ones_sb[:], u3rk[:], start=first, stop=last)

            for jc in range(JC):
                ktile = ldp.tile([P, CT, JBLK], bf16, tag="kld")
                nc.sync.dma_start(
                    ktile[:], kT_v[:, :, jc * JBLK:(jc + 1) * JBLK])
                vtile = ldp.tile([P, JSUB, C], bf16, tag="vld")
                nc.scalar.dma_start(vtile[:], vN_v[:, jc, :, :])
                if jc == 1:
                    nc.scalar.dma_start(wq[:], Wq_v[:])
                elif jc == 2:
                    nc.scalar.dma_start(wv[:], Wv_v[:])
                elif jc == 3:
                    nc.scalar.dma_start(wp[:], Wp_v[:])
                elif jc == 4:
                    nc.scalar.dma_start(bp_sb[:], bp_col[:])

                for jj in range(JSUB):
                    jsl = slice(jj * P, (jj + 1) * P)
                    kps = kpp.tile([P, C], f32, tag="kproj")
                    for ct in range(CT):
                        nc.tensor.matmul(
                            kps[:], ktile[:, ct, jsl], wk[:, ct, :],
                            start=(ct == 0), stop=(ct == CT - 1))
                    rlu = wkp.tile([P, C], bf16, tag="rlu")
                    nc.scalar.activation(rlu[:], kps[:], AF.Relu)
                    u2 = wkp.tile([P, C], bf16, tag="u2")
                    nc.gpsimd.tensor_tensor(u2[:], rlu[:], rlu[:], OP.mult)
                    S2 = smp.tile([P, 1], f32, tag="s2")
                    nc.vector.tensor_reduce(
                        out=S2[:], in_=u2[:], axis=mybir.AxisListType.X,
                        op=OP.add)
                    u3 = wkp.tile([P, C], bf16, tag="u3")
                    nc.vector.tensor_tensor(u3[:], u2[:], rlu[:], OP.mult)
                    # u6 is a dead scratch; S6 via ACT accumulator on even
                    # tiles, POOL square + DVE reduce on odd tiles
                    u6 = wkp.tile([P, C], bf16, tag="u6")
                    S6 = smp.tile([P, 1], f32, tag="s6")
                    if (jc * JSUB + jj) % 2 == 0:
                        nc.scalar.activation(
                            u6[:], u3[:], AF.Square, accum_out=S6[:])
                    else:
                        nc.gpsimd.tensor_tensor(u6[:], u3[:], u3[:], OP.mult)
                        nc.vector.tensor_reduce(
                            out=S6[:], in_=u6[:], axis=mybir.AxisListType.X,
                            op=OP.add)
                    rS6 = smp.tile([P, 1], f32, tag="rs6")
                    nc.vector.reciprocal(rS6[:], S6[:])
                    ratio = smp.tile([P, 1], f32, tag="ratio")
                    nc.vector.tensor_tensor(ratio[:], S2[:], rS6[:], OP.mult)
                    lnr = smp.tile([P, 1], f32, tag="lnr")
                    nc.scalar.activation(lnr[:], ratio[:], AF.Ln)
                    rk = smp.tile([P, 1], f32, tag="rk")
                    nc.scalar.activation(rk[:], lnr[:], AF.Exp, scale=0.5)
                    u3rk = u3rkp.tile([P, C], bf16, tag="u3rk")
                    nc.vector.tensor_scalar(
                        out=u3rk[:], in0=u3[:], scalar1=rk[:],
                        scalar2=None, op0=OP.mult)
                    pend.append((u3rk, vtile, jj))
                    if len(pend) > LAG:
                        emit_kvraw()
                if jc == JC - 1:
                    dma_q(0)
            # qproj(0)/(1) on the kproj psum pool cover the kvraw drain
            # and the transition matmul chain
            u3qs_all = {}
            u3qs_all[0] = emit_qproj(0, kpp, "kproj")
            while pend:
                emit_kvraw()
            dma_q(1)
            u3qs_all[1] = emit_qproj(1, kpp, "kproj")

        # ---------- transition ----------
        # KVrawT psum -> sbuf bf16 (split across ACT/DVE engines)
        nc.scalar.activation(KVr_sb[:, 0, :], kvr[0][:], AF.Identity)
        nc.scalar.activation(KVr_sb[:, 1, :], kvr[1][:], AF.Identity)
        nc.vector.tensor_copy(KVr_sb[:, 2, :], kvr[2][:])
        nc.vector.tensor_copy(KVr_sb[:, 3, :], kvr[3][:])
        # ksum row -> DRAM bounce -> column layout
        nc.scalar.activation(krow_sb[:], ksum_ps[:], AF.Identity)
        nc.sync.dma_start(ksb[:], krow_sb[:])
        nc.sync.dma_start(kcol_sb[:], ksb_colv[:])
        kvstack.close()

        # phase-2 psum pool (kvps banks just freed)
        p2 = ExitStack()
        qpsp = p2.enter_context(
            tc.tile_pool(name="qps", bufs=3, space="PSUM"))

        # transition matmuls: per-head kv_bdT then M = kv_bd @ Wp
        with ExitStack() as tr:
            kvbp = tr.enter_context(
                tc.tile_pool(name="kvbd", bufs=1, space="PSUM"))
            mpp = tr.enter_context(
                tc.tile_pool(name="mps", bufs=1, space="PSUM"))
            kvbd_ps = kvbp.tile([P, CT, HD], f32, tag="kvbdps")
            m_ps = [mpp.tile([P, C], f32, tag=f"mps{t}", name=f"mps{t}")
                    for t in range(CT)]
            for hh in range(NH):
                ct, half = hh // 2, hh % 2
                sl = slice(half * HD, half * HD + HD)
                hs = slice(hh * HD, hh * HD + HD)
                for rt in range(CT):
                    nc.tensor.matmul(
                        kvbd_ps[sl, ct, :], wv[:, rt, hs], KVr_sb[:, rt, hs],
                        start=(rt == 0), stop=(rt == CT - 1))
            for ct in range(CT):
                nc.vector.tensor_copy(kvbd_sb[:, ct, :], kvbd_ps[:, ct, :])
            for hh in range(NH):
                ct, half = hh // 2, hh % 2
                sl = slice(half * HD, half * HD + HD)
                nc.tensor.matmul(
                    m_ps[ct][sl, :], kvbd_sb[sl, ct, :], wp[sl, ct, :],
                    start=True, stop=True)
            nc.scalar.activation(M_sb[:, 0, :], m_ps[0][:], AF.Identity)
            nc.scalar.activation(M_sb[:, 1, :], m_ps[1][:], AF.Identity)
            nc.vector.tensor_copy(M_sb[:, 2, :], m_ps[2][:])
            nc.vector.tensor_copy(M_sb[:, 3, :], m_ps[3][:])

        # m8rep_diag: blockdiag(mask) * ksum per partition, bf16
        for ct in range(CT):
            nc.gpsimd.memset(m8rep[:, ct, :], 0.0)
            for half in range(2):
                sl = slice(half * HD, half * HD + HD)
                nc.vector.tensor_scalar(
                    out=m8rep[sl, ct, sl], in0=ones_hd[sl, :],
                    scalar1=kcol_sb[sl, ct:ct + 1], scalar2=None,
                    op0=OP.mult)

        # ================= PHASE 2 main loop =================
        trpp = p2.enter_context(
            tc.tile_pool(name="trep", bufs=2, space="PSUM"))
        opsp = p2.enter_context(
            tc.tile_pool(name="ops", bufs=3, space="PSUM"))

        def attn_pre(u3qs):
            gu3qs = []
            for nt in range(CT):
                trep = trpp.tile([P, IBLK], f32, tag="trep", name="trep")
                nc.tensor.matmul(
                    trep[:], m8rep[:, nt, :], u3qs[nt][:],
                    start=True, stop=True)
                g = gwp.tile([P, IBLK], f32, tag="g", name="g")
                nc.vector.reciprocal_approx_fast(out=g[:], in_=trep[:])
                gu = gup.tile([P, IBLK], bf16, tag="gu", name="gu")
                if nt < 2:
                    nc.vector.tensor_tensor(gu[:], u3qs[nt][:], g[:], OP.mult)
                else:
                    nc.gpsimd.tensor_tensor(gu[:], u3qs[nt][:], g[:], OP.mult)
                gu3qs.append(gu)
            return gu3qs

        def attn_out(gu3qs, ic):
            isl = slice(ic * IBLK, (ic + 1) * IBLK)
            for et in range(CT):
                ops_t = opsp.tile([P, IBLK], f32, tag="ops", name="ops")
                for nt in range(CT):
                    nc.tensor.matmul(
                        ops_t[:], M_sb[:, nt, et * P:(et + 1) * P],
                        gu3qs[nt][:],
                        start=(nt == 0), stop=(nt == CT - 1))
                out_sb = osp.tile([P, IBLK], f32, tag="osb", name="osb")
                if et % 2 == 0:
                    nc.scalar.activation(
                        out_sb[:], ops_t[:], AF.Identity,
                        bias=bp_sb[:, et:et + 1])
                else:
                    nc.vector.tensor_scalar(
                        out=out_sb[:], in0=ops_t[:],
                        scalar1=bp_sb[:, et:et + 1], scalar2=None,
                        op0=OP.add)
                nc.sync.dma_start(outT_v[:, et, isl], out_sb[:])

        gu_all = {}
        gu_all[0] = attn_pre(u3qs_all.pop(0))
        gu_all[1] = attn_pre(u3qs_all.pop(1))
        for ic in range(ICN):
            if ic + 2 < ICN:
                dma_q(ic + 2)
                u3qs_all[ic + 2] = emit_qproj(ic + 2, qpsp, "qps")
                gu_all[ic + 2] = attn_pre(u3qs_all.pop(ic + 2))
            attn_out(gu_all.pop(ic), ic)
        p2.close()

    nc.compile()
    return nc


def _get_nc():
    key = "nc"
    if key not in _CACHE:
        _CACHE[key] = _build_nc()
    return _CACHE[key]


def _prepare_in_maps(query, key_in, value, Wq, Wk, Wv, Wp, bp, scale):
    import ml_dtypes
    BF = ml_dtypes.bfloat16

    query = np.asarray(query, np.float32)
    key_in = np.asarray(key_in, np.float32)
    value = np.asarray(value, np.float32)
    Wq = np.asarray(Wq, np.float32)
    Wk = np.asarray(Wk, np.float32)
    Wv = np.asarray(Wv, np.float32)
    Wp = np.asarray(Wp, np.float32)
    bp = np.asarray(bp, np.float32)
    scale = np.asarray(scale, np.float32)

    B = query.shape[0]
    assert B == NCORES and query.shape[1] == N and query.shape[2] == C

    def rnd(a):
        return np.ascontiguousarray(np.asarray(a, np.float32).astype(BF))

    # softplus(scale) folded into Wq/Wk columns (relu(x)/s == relu(x/s), s>0)
    s = np.log1p(np.exp(np.float64(scale.reshape(C)))).astype(np.float32)
    inv_s = (1.0 / s).astype(np.float32)
    Wq_s = rnd(Wq * inv_s[None, :])
    Wk_s = rnd(Wk * inv_s[None, :])
    Wv_r = rnd(Wv)
    Wp_r = rnd(Wp)
    bp_col = np.ascontiguousarray(bp.reshape(CT, P).T)

    in_maps = []
    for b in range(B):
        in_maps.append({
            "qT": rnd(query[b].T),
            "kT": rnd(key_in[b].T),
            "vN": rnd(value[b]),
            "Wq": Wq_s, "Wk": Wk_s, "Wv": Wv_r, "Wp": Wp_r,
            "bp_col": bp_col,
        })
    return in_maps


def kernel(query, key_in, value, Wq, Wk, Wv, Wp, bp, scale, H, W):
    from concourse.bass_utils import run_bass_kernel_spmd

    in_maps = _prepare_in_maps(
        query, key_in, value, Wq, Wk, Wv, Wp, bp, scale)
    nc = _get_nc()
    res = run_bass_kernel_spmd(nc, in_maps, list(range(NCORES)))
    out = np.empty((len(in_maps), N, C), np.float32)
    for b in range(len(in_maps)):
        out[b] = res.results[b]["outT"].T
    return out


if __name__ == "__main__":
    rng = np.random.default_rng(0)
    inputs = {
        "query": rng.standard_normal((8, N, C)).astype(np.float32),
        "key_in": rng.standard_normal((8, N, C)).astype(np.float32),
        "value": rng.standard_normal((8, N, C)).astype(np.float32),
        "Wq": (rng.standard_normal((C, C)) * 0.02).astype(np.float32),
        "Wk": (rng.standard_normal((C, C)) * 0.02).astype(np.float32),
        "Wv": (rng.standard_normal((C, C)) * 0.02).astype(np.float32),
        "Wp": (rng.standard_normal((C, C)) * 0.02).astype(np.float32),
        "bp": np.zeros((C,), np.float32),
        "scale": (rng.standard_normal((1, 1, C)) * 0.02).astype(np.float32),
        "H": 64, "W": 64,
    }
    out = kernel(**inputs)
    print("out", out.shape, out.dtype, float(np.abs(out).mean()))
